# revision 7
# baseline (speedup 1.0000x reference)
"""DepthToSpace (cell=4, 4 split groups) Trainium2 Bass kernel — v2.

Full input x: [8, 64, 256, 256] f32 -> output [8, 4, 1024, 1024] f32.
out[b, s, 4h+r, 4w+c] = x[b, 16s + 4r + c, h, w]

Sharding: data parallel over batch — core b handles x[b] (16.8 MB in/out).

Per-core plan (pure data movement, DMA-engine bound): partition
p = 32s + b owns split group s = p//32 and h-block b = p%32 (8 rows
h in [8b, 8b+8)), i.e. out[s] rows [32b, 32b+32).

  load   : X[p, ch, h3, w] = x[16s+ch, 8b+h3, w]
           -> 16 descriptors of 8KB per partition (vs 2KB in v1).
  shuffle: per half (64 partitions) and input row j: Y[p, r, w, c] =
           X[p, 4r+c, j, w]; split DVE/ACT along w.
  store  : Y -> y[s] rows 32b+4j .. 32b+4j+4 — one contiguous 16KB
           descriptor per partition.

SDMA descriptors round-robin over the 16 engines per dma_start
(measured), so any multiple-of-16 descriptor count is balanced.
Loads for all 4 splits are enqueued first; store chunks queue behind
them on the same HWDGE ring, so engines stream continuously:
loads ~36us, stores ~35us per engine, zero idle between.
"""

import sys

sys.path.insert(0, "/opt/trn_rl_repo")

import numpy as np

import concourse.bass as bass
import concourse.mybir as mybir
from concourse.bass_utils import run_bass_kernel_spmd

B, C, H, W = 8, 64, 256, 256
S = 4
CELL = 4  # sqrt(C // S)
CPG = C // S  # channels per group = 16
P = 128  # SBUF partitions
HB3 = 8  # h rows per partition block
NB = H // HB3  # 32 blocks per split
N_CORES = 8

NYB = 3  # Y chunk buffers (per half, independent partition ranges)
WSPLIT = 174  # DVE handles w < WSPLIT, ACT the rest


def build_program():
    nc = bass.Bass()
    x = nc.declare_dram_parameter("x", [C, H, W], mybir.dt.float32, isOutput=False)
    y = nc.declare_dram_parameter(
        "y", [S, H * CELL, W * CELL], mybir.dt.float32, isOutput=True
    )

    from contextlib import ExitStack

    with ExitStack() as ctx:
        X = ctx.enter_context(
            nc.sbuf_tensor("X", [P, CPG, HB3, W], mybir.dt.float32)
        )  # 128KiB/partition
        Yt = [
            ctx.enter_context(
                nc.sbuf_tensor(f"Y{i}", [P, CELL, W, CELL], mybir.dt.float32)
            )
            for i in range(NYB)
        ]  # 16KiB each
        ld = ctx.enter_context(nc.semaphore("ld"))
        st = ctx.enter_context(nc.semaphore("st"))
        shv = ctx.enter_context(nc.semaphore("shv"))
        sha = ctx.enter_context(nc.semaphore("sha"))
        block = ctx.enter_context(nc.Block())

        # x as [s, b, ch, h3, w]; per-(partition, ch) 8KB runs
        x_ap = x.rearrange("(s ch) (b h3) w -> s b ch h3 w", s=S, b=NB)
        # y as [s, b, j, r, w, c]; per-(partition, j) 16KB runs
        y_ap = y.rearrange("s (b j r) (w c) -> s b j r w c", b=NB, j=HB3, c=CELL)

        def copy_aps(h, j, slot, w0, w1):
            xr = X[:].rearrange("p (r c) h3 w -> p r c h3 w", r=CELL)
            src = xr[64 * h : 64 * h + 64, :, :, j, w0:w1]
            dst = Yt[slot][64 * h : 64 * h + 64, :, w0:w1, :].transpose([0, 1, 3, 2])
            return src, dst

        @block.sync
        def _(sync):
            for s in range(S):
                sync.dma_start(out=X[32 * s : 32 * s + 32], in_=x_ap[s]).then_inc(
                    ld, 16
                )
            for h in range(2):
                for j in range(HB3):
                    g = HB3 * h + j
                    sync.wait_ge(shv, g + 1)
                    sync.wait_ge(sha, g + 1)
                    slot = j % NYB
                    for s in (2 * h, 2 * h + 1):
                        sync.dma_start(
                            out=y_ap[s, :, j],
                            in_=Yt[slot][32 * s : 32 * s + 32],
                        ).then_inc(st, 16)
            sync.wait_ge(st, 32 * 2 * HB3)

        @block.vector
        def _(vector):
            for h in range(2):
                vector.wait_ge(ld, 32 * (h + 1))
                for j in range(HB3):
                    g = HB3 * h + j
                    if j >= NYB:
                        # Y[j % NYB] (this half) free once store (h, j-NYB) done
                        vector.wait_ge(st, 32 * (HB3 * h + j - NYB + 1))
                    src, dst = copy_aps(h, j, j % NYB, 0, WSPLIT)
                    vector.tensor_copy(out=dst, in_=src).then_inc(shv, 1)

        @block.scalar
        def _(scalar):
            for h in range(2):
                scalar.wait_ge(ld, 32 * (h + 1))
                for j in range(HB3):
                    g = HB3 * h + j
                    if j >= NYB:
                        scalar.wait_ge(st, 32 * (HB3 * h + j - NYB + 1))
                    src, dst = copy_aps(h, j, j % NYB, WSPLIT, W)
                    scalar.copy(out=dst, in_=src).then_inc(sha, 1)

    return nc


def run_sharded(x: np.ndarray, trace: bool = False):
    """Shard x over batch across 8 cores, run, gather. Returns (out, results)."""
    assert x.shape == (B, C, H, W), x.shape
    nc = build_program()
    in_maps = [{"x": np.ascontiguousarray(x[b])} for b in range(N_CORES)]
    res = run_bass_kernel_spmd(nc, in_maps, list(range(N_CORES)), trace=trace)
    out = np.stack([res.results[b]["y"] for b in range(N_CORES)], axis=0)
    return out.astype(x.dtype, copy=False), res


def kernel(**inputs: np.ndarray) -> np.ndarray:
    x = np.asarray(inputs["x"], dtype=np.float32)
    out, _ = run_sharded(x, trace=False)
    return out


# revision 10
# speedup vs baseline: 1.3372x; 1.3372x over previous
"""DepthToSpace (cell=4, 4 split groups) Trainium2 Bass kernel.

Full input x: [8, 64, 256, 256] f32 -> output [8, 4, 1024, 1024] f32.
out[b, s, 4h+r, 4w+c] = x[b, 16s + 4r + c, h, w]

Sharding: data parallel over batch — core b handles x[b] (16.8 MB in/out).

Per-core plan (pure data movement, memory-bound): partition p = h//2.
All DMAs issue from the Sync engine onto one HWDGE ring: the four loads
enqueue first (X triple buffered), stores queue strictly behind them,
so loads drain at full solo DMA bandwidth and stores drain
back-to-back afterward. Per split group s:
  load   : X[p, ch, h2, w] = x[16s+ch, 2p+h2, w]  (2KB DRAM runs)
  shuffle: Y[p, h2, r, w, c] = X[p, 4r+c, h2, w]  (strided copies),
           split DVE:ACT ~= 5:3 by elements to balance engine rates
  store  : Y -> y[s] rows 8p+4h2+r, cols 4w+c — a single fully
           contiguous 4MB region (32KB runs)
GPSIMD/SWDGE is deliberately unused (measured ~10% slower when issuing
DMA). The 4-byte-granularity interleave happens on-chip where strided
access is cheap; both DMA directions keep multi-KB contiguous runs.
"""

import sys

sys.path.insert(0, "/opt/trn_rl_repo")

import numpy as np

import concourse.bass as bass
import concourse.mybir as mybir
from concourse.bass_utils import run_bass_kernel_spmd

B, C, H, W = 8, 64, 256, 256
S = 4
CELL = 4  # sqrt(C // S)
CPG = C // S  # channels per group = 16
P = 128  # SBUF partitions
HB = H // P  # h rows per partition = 2
N_CORES = 8

NXB = 3  # X buffers
NYB = 3  # Y buffers

# Shuffle work units (h2, r_lo, r_hi) — DVE gets h2=0 all r + h2=1 r0;
# ACT gets h2=1 r1..r3.
DVE_UNITS = [(0, 0, 4), (1, 0, 1)]
ACT_UNITS = [(1, 1, 4)]


def build_program():
    nc = bass.Bass()
    x = nc.declare_dram_parameter("x", [C, H, W], mybir.dt.float32, isOutput=False)
    y = nc.declare_dram_parameter(
        "y", [S, H * CELL, W * CELL], mybir.dt.float32, isOutput=True
    )

    from contextlib import ExitStack

    with ExitStack() as ctx:
        sb = lambda name, shape: ctx.enter_context(
            nc.sbuf_tensor(name, shape, mybir.dt.float32)
        )
        sem = lambda name: ctx.enter_context(nc.semaphore(name))
        Xt = [sb(f"X{i}", [P, CPG, HB, W]) for i in range(NXB)]
        Yt = [sb(f"Y{i}", [P, HB, CELL, W, CELL]) for i in range(NYB)]
        inl = [sem(f"inl{i}") for i in range(NXB)]
        outs = [sem(f"outs{i}") for i in range(NYB)]
        shuf_v = sem("shuf_v")
        shuf_a = sem("shuf_a")
        block = ctx.enter_context(nc.Block())

        def load_ap(s):
            # x channels [16s, 16s+16); 2KB runs per (p, ch)
            return x[s * CPG : (s + 1) * CPG].rearrange(
                "ch (p h2) w -> p ch h2 w", h2=HB
            )

        def store_ap(s):
            # y[s] as [p, h2, r, w, c]: row = 8p+4h2+r, col = 4w+c.
            # Fully contiguous: 32KB per partition, one 4MB region.
            return y[s].rearrange(
                "(p h2 r) (w c) -> p h2 r w c", h2=HB, r=CELL, c=CELL
            )

        def copy_aps(Xb, Yb, h2, r_lo, r_hi):
            # src [p, r, c, w] == dst iteration (p, r, c, w)
            xr = Xb[:].rearrange("p (r c) h2 w -> p r c h2 w", r=CELL)
            src = xr[:, r_lo:r_hi, :, h2, :]
            dst = Yb[:, h2, r_lo:r_hi].transpose([0, 1, 3, 2])
            return src, dst

        n_dve = len(DVE_UNITS)
        n_act = len(ACT_UNITS)

        @block.sync
        def _(sync):
            for s in range(S):
                if s >= NXB:
                    # X[s%NXB] free once shuffle(s-NXB) fully done
                    sync.wait_ge(shuf_v, n_dve * (s - NXB + 1))
                    sync.wait_ge(shuf_a, n_act * (s - NXB + 1))
                sync.dma_start(out=Xt[s % NXB][:], in_=load_ap(s)).then_inc(
                    inl[s % NXB], 16
                )
            # Stores queue behind all loads on this ring, so loads drain
            # at full solo DMA bandwidth first. Each split's store is split
            # at partition 120: [0,120) gives engines 0-14 aligned 8-desc
            # blocks (engine 15 idle — it runs ~19% slow under profiling);
            # [120,128) routes port-15's stores through engines 0-7.
            for s in range(S):
                sync.wait_ge(shuf_v, n_dve * (s + 1))
                sync.wait_ge(shuf_a, n_act * (s + 1))
                sap = store_ap(s)
                sync.dma_start(out=sap[0:120], in_=Yt[s % NYB][0:120]).then_inc(
                    outs[s % NYB], 16
                )
                sync.dma_start(out=sap[120:128], in_=Yt[s % NYB][120:128]).then_inc(
                    outs[s % NYB], 16
                )
            for b in range(NYB):
                sync.wait_ge(outs[b], 32 * (S // NYB + (1 if b < S % NYB else 0)))

        @block.vector
        def _(vector):
            for s in range(S):
                vector.wait_ge(inl[s % NXB], 16 * (s // NXB + 1))
                if s >= NYB:
                    vector.wait_ge(outs[s % NYB], 32 * (s // NYB))
                for h2, r_lo, r_hi in DVE_UNITS:
                    src, dst = copy_aps(Xt[s % NXB], Yt[s % NYB], h2, r_lo, r_hi)
                    vector.tensor_copy(out=dst, in_=src).then_inc(shuf_v, 1)

        @block.scalar
        def _(scalar):
            for s in range(S):
                scalar.wait_ge(inl[s % NXB], 16 * (s // NXB + 1))
                if s >= NYB:
                    scalar.wait_ge(outs[s % NYB], 32 * (s // NYB))
                for h2, r_lo, r_hi in ACT_UNITS:
                    src, dst = copy_aps(Xt[s % NXB], Yt[s % NYB], h2, r_lo, r_hi)
                    scalar.copy(out=dst, in_=src).then_inc(shuf_a, 1)

    return nc


def run_sharded(x: np.ndarray, trace: bool = False):
    """Shard x over batch across 8 cores, run, gather. Returns (out, results)."""
    assert x.shape == (B, C, H, W), x.shape
    nc = build_program()
    in_maps = [{"x": np.ascontiguousarray(x[b])} for b in range(N_CORES)]
    res = run_bass_kernel_spmd(nc, in_maps, list(range(N_CORES)), trace=trace)
    out = np.stack([res.results[b]["y"] for b in range(N_CORES)], axis=0)
    return out.astype(x.dtype, copy=False), res


def kernel(**inputs: np.ndarray) -> np.ndarray:
    x = np.asarray(inputs["x"], dtype=np.float32)
    out, _ = run_sharded(x, trace=False)
    return out



# revision 12
# speedup vs baseline: 1.6141x; 1.2071x over previous
"""DepthToSpace (cell=4, 4 split groups) Trainium2 Bass kernel.

Full input x: [8, 64, 256, 256] f32 -> output [8, 4, 1024, 1024] f32.
out[b, s, 4h+r, 4w+c] = x[b, 16s + 4r + c, h, w]

Sharding: data parallel over batch — core b handles x[b] (16.8 MB in/out).

Per-core plan (pure data movement, memory-bound): partition p = h//2.
All DMAs issue from the Sync engine onto one HWDGE ring: the four loads
enqueue first (X triple buffered), stores queue strictly behind them,
so loads drain at full solo DMA bandwidth and stores drain
back-to-back afterward. Per split group s:
  load   : X[p, ch, h2, w] = x[16s+ch, 2p+h2, w]  (2KB DRAM runs)
  shuffle: Y[p, h2, r, w, c] = X[p, 4r+c, h2, w]  (strided copies),
           split DVE:ACT ~= 5:3 by elements to balance engine rates
  store  : Y -> y[s] rows 8p+4h2+r, cols 4w+c — a single fully
           contiguous 4MB region (32KB runs)
GPSIMD/SWDGE is deliberately unused (measured ~10% slower when issuing
DMA). The 4-byte-granularity interleave happens on-chip where strided
access is cheap; both DMA directions keep multi-KB contiguous runs.
"""

import sys

sys.path.insert(0, "/opt/trn_rl_repo")

import numpy as np

import concourse.bass as bass
import concourse.mybir as mybir
from concourse.bass_utils import run_bass_kernel_spmd

B, C, H, W = 8, 64, 256, 256
S = 4
CELL = 4  # sqrt(C // S)
CPG = C // S  # channels per group = 16
P = 128  # SBUF partitions
HB = H // P  # h rows per partition = 2
N_CORES = 8

NXB = 2  # X buffers
NYB = 4  # Y buffers: with 4, shuffle s3 needs no store completion first —
# the store-s0 -> shuffle-s3 -> store-s3 serial tail of NYB=3 disappears.

# Shuffle work units (h2, r_lo, r_hi) — DVE gets h2=0 all r + h2=1 r0;
# ACT gets h2=1 r1..r3.
DVE_UNITS = [(0, 0, 4), (1, 0, 1)]
ACT_UNITS = [(1, 1, 4)]


def build_program():
    nc = bass.Bass()
    x = nc.declare_dram_parameter("x", [C, H, W], mybir.dt.float32, isOutput=False)
    y = nc.declare_dram_parameter(
        "y", [S, H * CELL, W * CELL], mybir.dt.float32, isOutput=True
    )

    from contextlib import ExitStack

    with ExitStack() as ctx:
        sb = lambda name, shape: ctx.enter_context(
            nc.sbuf_tensor(name, shape, mybir.dt.float32)
        )
        sem = lambda name: ctx.enter_context(nc.semaphore(name))
        Xt = [sb(f"X{i}", [P, CPG, HB, W]) for i in range(NXB)]
        Yt = [sb(f"Y{i}", [P, HB, CELL, W, CELL]) for i in range(NYB)]
        inl = [sem(f"inl{i}") for i in range(NXB)]
        outs = [sem(f"outs{i}") for i in range(NYB)]
        shuf_v = sem("shuf_v")
        shuf_a = sem("shuf_a")
        block = ctx.enter_context(nc.Block())

        def load_ap(s):
            # x channels [16s, 16s+16); 2KB runs per (p, ch)
            return x[s * CPG : (s + 1) * CPG].rearrange(
                "ch (p h2) w -> p ch h2 w", h2=HB
            )

        def store_ap(s):
            # y[s] as [p, h2, r, w, c]: row = 8p+4h2+r, col = 4w+c.
            # Fully contiguous: 32KB per partition, one 4MB region.
            return y[s].rearrange(
                "(p h2 r) (w c) -> p h2 r w c", h2=HB, r=CELL, c=CELL
            )

        def copy_aps(Xb, Yb, h2, r_lo, r_hi):
            # src [p, r, c, w] == dst iteration (p, r, c, w)
            xr = Xb[:].rearrange("p (r c) h2 w -> p r c h2 w", r=CELL)
            src = xr[:, r_lo:r_hi, :, h2, :]
            dst = Yb[:, h2, r_lo:r_hi].transpose([0, 1, 3, 2])
            return src, dst

        n_dve = len(DVE_UNITS)
        n_act = len(ACT_UNITS)

        @block.sync
        def _(sync):
            for s in range(S):
                if s >= NXB:
                    # X[s%NXB] free once shuffle(s-NXB) fully done
                    sync.wait_ge(shuf_v, n_dve * (s - NXB + 1))
                    sync.wait_ge(shuf_a, n_act * (s - NXB + 1))
                sync.dma_start(out=Xt[s % NXB][:], in_=load_ap(s)).then_inc(
                    inl[s % NXB], 16
                )
            # Stores queue behind all loads on this ring, so loads drain
            # at full solo DMA bandwidth first.
            for s in range(S):
                sync.wait_ge(shuf_v, n_dve * (s + 1))
                sync.wait_ge(shuf_a, n_act * (s + 1))
                sync.dma_start(out=store_ap(s), in_=Yt[s % NYB][:]).then_inc(
                    outs[s % NYB], 16
                )
            for b in range(NYB):
                sync.wait_ge(outs[b], 16 * (S // NYB + (1 if b < S % NYB else 0)))

        @block.vector
        def _(vector):
            for s in range(S):
                vector.wait_ge(inl[s % NXB], 16 * (s // NXB + 1))
                if s >= NYB:
                    vector.wait_ge(outs[s % NYB], 16 * (s // NYB))
                for h2, r_lo, r_hi in DVE_UNITS:
                    src, dst = copy_aps(Xt[s % NXB], Yt[s % NYB], h2, r_lo, r_hi)
                    vector.tensor_copy(out=dst, in_=src).then_inc(shuf_v, 1)

        @block.scalar
        def _(scalar):
            for s in range(S):
                scalar.wait_ge(inl[s % NXB], 16 * (s // NXB + 1))
                if s >= NYB:
                    scalar.wait_ge(outs[s % NYB], 16 * (s // NYB))
                for h2, r_lo, r_hi in ACT_UNITS:
                    src, dst = copy_aps(Xt[s % NXB], Yt[s % NYB], h2, r_lo, r_hi)
                    scalar.copy(out=dst, in_=src).then_inc(shuf_a, 1)

    return nc


def run_sharded(x: np.ndarray, trace: bool = False):
    """Shard x over batch across 8 cores, run, gather. Returns (out, results)."""
    assert x.shape == (B, C, H, W), x.shape
    nc = build_program()
    in_maps = [{"x": np.ascontiguousarray(x[b])} for b in range(N_CORES)]
    res = run_bass_kernel_spmd(nc, in_maps, list(range(N_CORES)), trace=trace)
    out = np.stack([res.results[b]["y"] for b in range(N_CORES)], axis=0)
    return out.astype(x.dtype, copy=False), res


def kernel(**inputs: np.ndarray) -> np.ndarray:
    x = np.asarray(inputs["x"], dtype=np.float32)
    out, _ = run_sharded(x, trace=False)
    return out

